# revision 1
# baseline (speedup 1.0000x reference)
"""Trainium2 Bass kernel for DiffVorticeSketchRender.

Sharding: 8 cores = 4 batches x 2 H-halves (64 rows each + 3-4 row halos).
Device layout: [D=128 partitions, H slices, W free] everywhere.
- curl + fdiffs: PSUM-accumulated matmuls with +/-I and D-difference band
  matrices (H/W shifts via shifted rhs access patterns, W edge handled by a
  host-extrapolated 129th column, D edge inside the band matrix).
- 3D gaussian smooth (separable): 7 accumulated matmuls fuse the D-conv
  (band matrix) with the H-conv (shifted slice windows), then 7 accumulated
  identity matmuls with shifted W windows for the W-conv.
- depth flip + cumsum: one suffix-sum triangular matmul.
- transmittance/integration: exp on ScalarE, band-matrix matmul for the
  trapezoid coefficients, ones/e127 reduction matmuls. All fp32r, N>=256.
"""

import numpy as np

import concourse.bacc as bacc
import concourse.bass as bass
import concourse.mybir as mybir
import concourse.tile as tile
from concourse.bass_utils import run_bass_kernel_spmd

F32 = mybir.dt.float32
F32R = mybir.dt.float32r
AL = mybir.AluOpType
AF = mybir.ActivationFunctionType

KHS, SIGMA, C = 3, 1.6, 20.0


def _gauss1d():
    size = 2 * KHS + 1
    g = np.arange(size, dtype=np.float64) - (size - 1) / 2.0
    g = np.exp(-((g / SIGMA) ** 2) / 2.0) / (SIGMA * np.sqrt(2.0 * np.pi))
    return (g / g.sum()).astype(np.float32)


GK = _gauss1d()


def _const_mats():
    mdz = np.zeros((128, 128), np.float32)
    for d in range(127):
        mdz[d, d] = -1.0
        mdz[d, d + 1] = 1.0
    mdz[127, 126] = -1.0
    mdz[127, 127] = 1.0

    bd = np.zeros((128, 128), np.float32)
    for dp in range(128):
        for k in range(7):
            d = dp + k - 3
            if 0 <= d < 128:
                bd[dp, d] = GK[k]

    mc = np.zeros((128, 128), np.float32)
    mc[0, 0], mc[0, 1] = -0.5, 0.5
    for k in range(1, 127):
        mc[k, k - 1], mc[k, k + 1] = -0.5, 0.5
    mc[127, 126], mc[127, 127] = -0.5, -0.5

    eye = np.eye(128, dtype=np.float32)
    kbd = np.stack([(GK[k] * bd).T for k in range(7)], axis=1)  # [128,7,128] lhsT, D+H pass
    ki = np.stack([GK[k] * eye for k in range(7)], axis=1)      # [128,7,128] lhsT, W pass
    suf = (np.arange(128)[:, None] >= np.arange(128)[None, :]).astype(np.float32)
    red = np.zeros((128, 2), np.float32)
    red[:, 0] = 1.0
    red[127, 1] = 1.0
    return {
        "KBD": kbd, "KI": ki, "CIP": eye, "CIN": -eye,
        "MDZT": mdz.T.copy(), "MDZTN": (-mdz.T).copy(),
        "SUF": suf, "MCT": mc.T.copy(), "RED": red,
    }


def _curl_groups():
    gs = []
    s0 = 0
    while s0 < 70:
        cnt = min(4, 70 - s0)
        gs.append((s0, cnt))
        s0 += cnt
    return gs


def build_program():
    nc = bacc.Bacc("TRN2", target_bir_lowering=False, debug=False)

    d_in = nc.dram_tensor("d_in", [128, 70, 128], F32R, kind="ExternalInput")
    v_in = nc.dram_tensor("v_in", [3, 128, 71, 129], F32R, kind="ExternalInput")
    m0_in = nc.dram_tensor("m0_in", [128, 3, 128], F32, kind="ExternalInput")
    m1_in = nc.dram_tensor("m1_in", [128, 3, 128], F32, kind="ExternalInput")
    cm = _const_mats()
    c_in = {}
    for name, arr in cm.items():
        c_in[name] = nc.dram_tensor(f"c_{name}", list(arr.shape), F32R,
                                    kind="ExternalInput")
    zpad_in = nc.dram_tensor("zpad", [128, 64, 6], F32R, kind="ExternalInput")
    out_t = nc.dram_tensor("out", [1, 8192], F32, kind="ExternalOutput")

    with tile.TileContext(nc) as tc:
        with tc.tile_pool(name="const", bufs=1) as cpool, \
             tc.tile_pool(name="vols", bufs=1) as vol:
            ct = {}
            for name, arr in cm.items():
                t = cpool.tile(list(arr.shape), F32R, tag=f"c_{name}")
                nc.sync.dma_start(t[:], c_in[name][:])
                ct[name] = t
            m0t = cpool.tile([128, 3, 128], F32, tag="m0")
            m1t = cpool.tile([128, 3, 128], F32, tag="m1")
            nc.sync.dma_start(m0t[:], m0_in[:])
            nc.sync.dma_start(m1t[:], m1_in[:])

            vn = vol.tile([128, 70, 128], F32R, tag="vn")

            # ---- stage 1: curl + |curl|^2 (scoped so v frees after) ----
            with tc.tile_pool(name="vdata", bufs=1) as vp, \
                 tc.tile_pool(name="sq", bufs=4) as sqp, \
                 tc.tile_pool(name="cpsum", bufs=2,
                              space=bass.MemorySpace.PSUM) as cps:
                du = vp.tile([128, 71, 129], F32R, tag="du")
                dv = vp.tile([128, 71, 129], F32R, tag="dv")
                dw = vp.tile([128, 71, 129], F32R, tag="dw")
                # chunk channel loads so early curl groups overlap the DMA
                for a, b in ((0, 6), (6, 13), (13, 25), (25, 37),
                             (37, 49), (49, 61), (61, 71)):
                    nc.sync.dma_start(du[:, a:b, :], v_in[0, :, a:b, :])
                    nc.sync.dma_start(dv[:, a:b, :], v_in[1, :, a:b, :])
                    nc.sync.dma_start(dw[:, a:b, :], v_in[2, :, a:b, :])

                for (s0, cnt) in _curl_groups():
                    n = cnt * 128
                    pcu = cps.tile([128, cnt, 128], F32, tag="pcu")
                    pcv = cps.tile([128, cnt, 128], F32, tag="pcv")
                    pcw = cps.tile([128, cnt, 128], F32, tag="pcw")
                    nc.tensor.matmul(pcu[:], ct["CIP"][:],
                                     dw[:, s0 + 1:s0 + 1 + cnt, 0:128],
                                     start=True, stop=False)
                    nc.tensor.matmul(pcu[:], ct["CIN"][:],
                                     dw[:, s0:s0 + cnt, 0:128],
                                     start=False, stop=False)
                    nc.tensor.matmul(pcu[:], ct["MDZTN"][:],
                                     dv[:, s0:s0 + cnt, 0:128], start=False, stop=True)

                    nc.tensor.matmul(pcv[:], ct["MDZT"][:],
                                     du[:, s0:s0 + cnt, 0:128], start=True, stop=False)
                    nc.tensor.matmul(pcv[:], ct["CIN"][:],
                                     dw[:, s0:s0 + cnt, 1:129],
                                     start=False, stop=False)
                    nc.tensor.matmul(pcv[:], ct["CIP"][:],
                                     dw[:, s0:s0 + cnt, 0:128], start=False, stop=True)

                    nc.tensor.matmul(pcw[:], ct["CIP"][:],
                                     dv[:, s0:s0 + cnt, 1:129], start=True, stop=False)
                    nc.tensor.matmul(pcw[:], ct["CIN"][:],
                                     dv[:, s0:s0 + cnt, 0:128],
                                     start=False, stop=False)
                    nc.tensor.matmul(pcw[:], ct["CIN"][:],
                                     du[:, s0 + 1:s0 + 1 + cnt, 0:128],
                                     start=False, stop=False)
                    nc.tensor.matmul(pcw[:], ct["CIP"][:],
                                     du[:, s0:s0 + cnt, 0:128], start=False, stop=True)

                    squ = sqp.tile([128, cnt, 128], F32, tag="squ")
                    sqv = sqp.tile([128, cnt, 128], F32, tag="sqv")
                    sqw = sqp.tile([128, cnt, 128], F32, tag="sqw")
                    nc.scalar.activation(squ[:], pcu[:], AF.Square)
                    nc.scalar.activation(sqv[:], pcv[:], AF.Square)
                    nc.scalar.activation(sqw[:], pcw[:], AF.Square)
                    tsum = sqp.tile([128, cnt, 128], F32, tag="tsum")
                    nc.vector.tensor_add(tsum[:], squ[:], sqv[:])
                    nc.vector.tensor_add(vn[:, s0:s0 + cnt, :],
                                         tsum[:], sqw[:])

            # mask out-of-range boundary slices, then sqrt in place
            nc.vector.tensor_mul(vn[:, 0:3, :], vn[:, 0:3, :], m0t[:])
            nc.vector.tensor_mul(vn[:, 67:70, :], vn[:, 67:70, :], m1t[:])
            for a, b in ((0, 20), (20, 37), (37, 54), (54, 70)):
                nc.scalar.activation(vn[:, a:b, :], vn[:, a:b, :], AF.Sqrt)

            # ---- stage 2/3: the two 3D smooths ----
            smp_cm = tc.tile_pool(name="smoothp", bufs=1)
            smp = smp_cm.__enter__()
            s1 = smp.tile([128, 64, 134], F32R, tag="s1")
            s1d = smp.tile([128, 64, 134], F32R, tag="s1d")
            for t in (s1, s1d):
                nc.sync.dma_start(t[:, :, 0:3], zpad_in[:, :, 0:3])
                nc.sync.dma_start(t[:, :, 131:134], zpad_in[:, :, 3:6])
            vns = smp.tile([128, 64, 128], F32R, tag="vns")
            dd = smp.tile([128, 70, 128], F32R, tag="dd")
            nc.sync.dma_start(dd[:], d_in[:])
            ds = smp.tile([128, 64, 128], F32R, tag="dd")

            def smooth(src, dst, s1):
                with tc.tile_pool(name="spsum", bufs=3,
                                  space=bass.MemorySpace.PSUM) as sps:
                    for go in range(16):
                        g4 = go * 4
                        p1 = sps.tile([128, 4, 128], F32, tag="p1")
                        for k in range(7):
                            nc.tensor.matmul(p1[:], ct["KBD"][:, k, :],
                                             src[:, g4 + k:g4 + k + 4, :],
                                             start=(k == 0), stop=(k == 6))
                        if go % 2 == 0:
                            nc.scalar.copy(s1[:, g4:g4 + 4, 3:131], p1[:])
                        else:
                            nc.vector.tensor_copy(s1[:, g4:g4 + 4, 3:131],
                                                  p1[:])
                    for go in range(16):
                        g4 = go * 4
                        p2 = sps.tile([128, 4, 128], F32, tag="p2")
                        for k in range(7):
                            nc.tensor.matmul(p2[:], ct["KI"][:, k, :],
                                             s1[:, g4:g4 + 4, k:k + 128],
                                             start=(k == 0), stop=(k == 6))
                        if go % 2 == 0:
                            nc.vector.tensor_copy(dst[:, g4:g4 + 4, :], p2[:])
                        else:
                            nc.scalar.copy(dst[:, g4:g4 + 4, :], p2[:])

            smooth(vn, vns, s1)
            smooth(dd, ds, s1d)

            # ---- stage 4: transmittance + trapezoid integration ----
            ivsb = smp.tile([1, 8192], F32, tag="s1")
            with tc.tile_pool(name="post", bufs=3) as pp, \
                 tc.tile_pool(name="ppsum", bufs=2,
                              space=bass.MemorySpace.PSUM) as pps:
                for cc in range(16):
                    g4 = cc * 4
                    ps = pps.tile([128, 4, 128], F32, tag="ps")
                    nc.tensor.matmul(ps[:], ct["SUF"][:], ds[:, g4:g4 + 4, :],
                                     start=True, stop=True)
                    ec = pp.tile([128, 4, 128], F32R, tag="ec")
                    bc = pp.tile([128, 4, 128], F32R, tag="bc")
                    nc.scalar.activation(ec[:], ps[:], AF.Exp, scale=-C)
                    nc.scalar.activation(bc[:], ps[:], AF.Copy, bias=1.0,
                                         scale=C)
                    nc.vector.tensor_mul(bc[:], bc[:], ec[:])
                    pc2 = pps.tile([128, 4, 128], F32, tag="pc2")
                    nc.tensor.matmul(pc2[:], ct["MCT"][:], bc[:],
                                     start=True, stop=True)
                    pchunk = pp.tile([128, 4, 128], F32R, tag="pchunk")
                    nc.vector.tensor_mul(pchunk[:], pc2[:],
                                         vns[:, g4:g4 + 4, :])
                    piv = pps.tile([1, 512], F32, tag="piv")
                    nc.tensor.matmul(piv[:], ct["RED"][:, 0:1], pchunk[:],
                                     start=True, stop=False)
                    nc.tensor.matmul(piv[:], ct["RED"][:, 1:2],
                                     vns[:, g4:g4 + 4, :], start=False, stop=True)
                    nc.vector.tensor_scalar_min(
                        ivsb[0:1, cc * 512:(cc + 1) * 512], piv[:], 1.0)
                nc.vector.tensor_scalar_max(ivsb[:], ivsb[:], 0.0)
                nc.sync.dma_start(out_t[:], ivsb[:])
            smp_cm.__exit__(None, None, None)

    nc.compile()
    return nc


def host_prepare(d_np, v_np):
    cores = []
    zeros3 = np.zeros((128, 3, 128), np.float32)
    ones3 = np.ones((128, 3, 128), np.float32)
    vext = np.zeros((3, 128, 135, 129), np.float32)
    cm = _const_mats()
    for c in range(8):
        b, hh = c // 2, c % 2
        h0 = 64 * hh
        dpad = np.zeros((128, 70, 128), np.float32)
        lo, hi = h0 - 3, h0 + 67
        src_lo, src_hi = max(lo, 0), min(hi, 128)
        dpad[:, (src_lo - lo):(src_hi - lo), :] = \
            d_np[b, 0, :, src_lo:src_hi, :]
        vext[:] = 0.0
        vext[:, :, 3:131, 0:128] = v_np[b]
        vext[:, :, 131, 0:128] = 2 * v_np[b, :, :, 127, :] - v_np[b, :, :, 126, :]
        vext[:, :, :, 128] = 2 * vext[:, :, :, 127] - vext[:, :, :, 126]
        vin = np.ascontiguousarray(vext[:, :, h0:h0 + 71, :])
        m = {
            "d_in": dpad, "v_in": vin,
            "zpad": np.zeros((128, 64, 6), np.float32),
            "m0_in": zeros3 if hh == 0 else ones3,
            "m1_in": zeros3 if hh == 1 else ones3,
        }
        for name, arr in cm.items():
            m[f"c_{name}"] = arr
        cores.append(m)
    return cores


_NC = None


def kernel(d, v):
    global _NC
    d = np.asarray(d, np.float32)
    v = np.asarray(v, np.float32)
    if _NC is None:
        _NC = build_program()
    in_maps = host_prepare(d, v)
    res = run_bass_kernel_spmd(_NC, in_maps, list(range(8)))
    out = np.zeros((4, 1, 128, 128), np.float32)
    for c in range(8):
        b, hh = c // 2, c % 2
        out[b, 0, 64 * hh:64 * hh + 64, :] = \
            res.results[c]["out"].reshape(64, 128)
    return out



# revision 29
# speedup vs baseline: 1.6449x; 1.6449x over previous
"""Trainium2 Bass kernel for DiffVorticeSketchRender (v2, bf16).

Sharding: 8 cores = 4 batches x 2 H-halves (64 rows each + halos).
Device layout: [D=128 partitions, H slices, W free] everywhere, all bf16
(inputs converted on host; PSUM accumulation stays fp32).

- curl + |curl|^2: PSUM matmuls with +/-I and a D-difference band matrix;
  squares on ScalarE, sums on VectorE, sqrt chunks on ScalarE.
- 3D gaussian smooth of vn: 2 matmul passes using the symmetric-tap trick:
  pa_j = win_j + win_{6-j} on VectorE/GpSimd, then 4 accumulated matmuls
  (D-band fused into the H pass matrices).
- d path: pass1 fuses M_x = SUF@flip@BD (D-conv + depth flip + cumsum in a
  single 128x128 matrix) with the H taps; pass2 applies the W taps, giving
  the optical depth x directly in PSUM.
- transmittance/integration: exp on ScalarE straight from PSUM,
  bc=(x+1/C)*exp(-Cx) via scalar_tensor_tensor, trapezoid coefficients via
  one matmul with C*MC, front term folded in as a per-partition +e127 bias
  inside the pchunk STT, ones-matmul reduction into a persistent [16,512]
  PSUM tile, single fused clip.
"""

import numpy as np
import ml_dtypes

import concourse.bacc as bacc
import concourse.bass as bass
import concourse.mybir as mybir
import concourse.tile as tile
from concourse.bass_utils import run_bass_kernel_spmd

F32 = mybir.dt.float32
BF16 = mybir.dt.bfloat16
AL = mybir.AluOpType
AF = mybir.ActivationFunctionType
NPBF = ml_dtypes.bfloat16

KHS, SIGMA, C = 3, 1.6, 20.0

# ---- tuning knobs ----
SYM = {"d1": True, "v1": True, "v2": True, "d2": True}
# engine for each of the 3 sym pre-adds per group: "v"=vector, "p"=gpsimd
PREADD_ENG = {"d1": "vvv", "v1": "vvp", "v2": "vvp", "d2": "vvp"}
COPY_ENG = {"d1": "v", "v1": "s"}  # psum->sbuf copy engine per pass
SQW_DVE = set()


def _gauss1d():
    size = 2 * KHS + 1
    g = np.arange(size, dtype=np.float64) - (size - 1) / 2.0
    g = np.exp(-((g / SIGMA) ** 2) / 2.0) / (SIGMA * np.sqrt(2.0 * np.pi))
    return g / g.sum()  # per-axis normalized (3d product kernel sums to 1)


def _const_mats():
    gn = _gauss1d()
    eye = np.eye(128, dtype=np.float64)

    # BD: D-axis 'same' zero-padded conv: out[dp] = sum_k gn[k] * in[dp+k-3]
    bd = np.zeros((128, 128), np.float64)
    for dp in range(128):
        for k in range(7):
            d = dp + k - 3
            if 0 <= d < 128:
                bd[dp, d] = gn[k]

    # M_x = SUF @ flip @ BD : x_flip = cumsum(flip(BD @ v))
    mx = np.cumsum(bd[::-1, :], axis=0)

    # trapezoid coefficient matrix MC (consumes t stored in flipped order):
    # c[m] coefficient of vns[m]; c = MC @ t (+1 at m=127 added separately)
    mc = np.zeros((128, 128), np.float64)
    for m in range(1, 127):
        mc[m, 126 - m] += 0.5
        mc[m, 128 - m] -= 0.5
    mc[0, 126] += 0.5
    mc[0, 127] -= 0.5
    mc[127, 0] -= 0.5
    mc[127, 1] -= 0.5

    mdz = np.zeros((128, 128), np.float64)
    for d in range(127):
        mdz[d, d] = -1.0
        mdz[d, d + 1] = 1.0
    mdz[127, 126] = -1.0
    mdz[127, 127] = 1.0

    kbd = np.stack([(gn[j] * bd).T for j in range(4)], axis=1)   # [128,4,128]
    kmd = np.stack([(gn[j] * mx).T for j in range(4)], axis=1)   # [128,4,128]
    ki = np.stack([gn[j] * eye for j in range(4)], axis=1)       # [128,4,128]

    e127 = np.zeros((128, 1), np.float64)
    e127[127, 0] = 1.0

    out = {
        "KBD": kbd, "KMD": kmd, "KI": ki,
        "MDZT": mdz.T.copy(), "MDZTN": (-mdz.T).copy(),
        "IP": eye, "IN": -eye,
        "MCTC": (C * mc).T.copy(),
        "E127": e127, "ONESC": np.ones((128, 1), np.float64),
    }
    return {k: v.astype(NPBF) for k, v in out.items()}


def _curl_groups():
    gs = []
    s0 = 0
    while s0 < 70:
        cnt = min(4, 70 - s0)
        gs.append((s0, cnt))
        s0 += cnt
    return gs


CONSTS = _const_mats()


def build_program():
    nc = bacc.Bacc("TRN2", target_bir_lowering=False, debug=False)

    d_in = nc.dram_tensor("d_in", [128, 70, 128], BF16, kind="ExternalInput")
    v_in = nc.dram_tensor("v_in", [3, 128, 71, 129], BF16, kind="ExternalInput")
    m_in = nc.dram_tensor("m_in", [128, 4], F32, kind="ExternalInput")
    ccrl_in = nc.dram_tensor("ccrl_in", [128, 4, 128], BF16, kind="ExternalInput")
    csm_in = nc.dram_tensor("csm_in", [128, 12, 128], BF16, kind="ExternalInput")
    ctz_in = nc.dram_tensor("ctz_in", [128, 130], BF16, kind="ExternalInput")
    out_t = nc.dram_tensor("out", [16, 512], F32, kind="ExternalOutput")

    with tile.TileContext(nc) as tc:
        with tc.tile_pool(name="const", bufs=1) as cp, \
             tc.tile_pool(name="vol", bufs=1) as vol, \
             tc.tile_pool(name="sq", bufs=2) as sqp, \
             tc.tile_pool(name="pa", bufs=2) as pap, \
             tc.tile_pool(name="tz", bufs=4) as tzp:
            ccrl = cp.tile([128, 4, 128], BF16, tag="ccrl")
            csm = cp.tile([128, 12, 128], BF16, tag="csm")
            ctz = cp.tile([128, 130], BF16, tag="ctz")
            mt = cp.tile([128, 4], F32, tag="mt")

            vt = vol.tile([128, 3, 71, 129], BF16, tag="vt")
            dd = vol.tile([128, 70, 128], BF16, tag="dd")
            vn = vol.tile([128, 70, 128], BF16, tag="vn")
            s1 = vol.tile([128, 64, 134], BF16, tag="s1")
            s1d = vol.tile([128, 64, 134], BF16, tag="s1d")
            iv_sb = vol.tile([65, 6, 512], F32, tag="ivsb")

            # DMA order tuned so curl group 0 can start ~1.5us in and the
            # feed stays ahead of curl groups
            nc.sync.dma_start(ccrl[:], ccrl_in[:])
            for a, b in ((0, 4), (4, 8)):
                for c in (2, 1, 0):
                    nc.sync.dma_start(vt[:, c, a:b, :], v_in[c, :, a:b, :])
            nc.sync.dma_start(csm[:], csm_in[:])
            nc.sync.dma_start(dd[:, 0:10, :], d_in[:, 0:10, :])
            for c in (2, 1, 0):
                nc.sync.dma_start(vt[:, c, 8:13, :], v_in[c, :, 8:13, :])
            nc.sync.dma_start(dd[:, 10:40, :], d_in[:, 10:40, :])
            for c in range(3):
                nc.sync.dma_start(vt[:, c, 13:25, :], v_in[c, :, 13:25, :])
            nc.sync.dma_start(dd[:, 40:70, :], d_in[:, 40:70, :])
            for a, b in ((25, 37), (37, 49), (49, 61), (61, 71)):
                for c in range(3):
                    nc.sync.dma_start(vt[:, c, a:b, :], v_in[c, :, a:b, :])
            nc.sync.dma_start(ctz[:], ctz_in[:])
            nc.sync.dma_start(mt[:], m_in[:])
            for t in (s1, s1d):
                nc.gpsimd.memset(t[:, :, 0:3], 0.0)
                nc.gpsimd.memset(t[:, :, 131:134], 0.0)

            ct = {
                "IP": ccrl[:, 0, :], "IN": ccrl[:, 1, :],
                "MDZT": ccrl[:, 2, :], "MDZTN": ccrl[:, 3, :],
                "KBD": 0, "KMD": 4, "KI": 8,
                "MCTC": ctz[:, 0:128], "E127": ctz[:, 128:129],
                "ONESC": ctz[:, 129:130],
            }
            du, dv, dw = vt[:, 0], vt[:, 1], vt[:, 2]

            def preadd(dst, a, b, eng):
                if eng == "v":
                    nc.vector.tensor_add(dst, a, b)
                else:
                    nc.gpsimd.tensor_add(dst, a, b)

            def smooth_group(name, cc, src, kbase, psum_t, axis):
                """4-output-slice group cc: 7-tap conv via 4 matmuls (sym)."""
                kmat = lambda j: csm[:, kbase + j, :]
                g4 = 4 * cc
                if axis == "H":
                    win = lambda j: src[:, g4 + j:g4 + j + 4, :]
                else:
                    win = lambda j: src[:, g4:g4 + 4, j:j + 128]
                if SYM[name]:
                    engs = PREADD_ENG[name]
                    pas = []
                    for j in range(3):
                        pa = pap.tile([128, 4, 128], BF16, tag=f"pa{name}{j}",
                                      name="pa")
                        preadd(pa[:], win(j), win(6 - j), engs[j])
                        pas.append(pa)
                    for j in range(3):
                        nc.tensor.matmul(psum_t[:], kmat(j), pas[j][:],
                                         start=(j == 0), stop=False)
                    nc.tensor.matmul(psum_t[:], kmat(3), win(3),
                                     start=False, stop=True)
                else:
                    for j in range(7):
                        jj = min(j, 6 - j)
                        nc.tensor.matmul(psum_t[:], kmat(jj), win(j),
                                         start=(j == 0), stop=(j == 6))

            def copyout(dst, src, eng):
                if eng == "s":
                    nc.scalar.copy(dst, src)
                else:
                    nc.vector.tensor_copy(dst, src)

            # ------------- interleaved schedule -------------
            GROUPS = _curl_groups()
            bcv = vol.tile([128, 64, 128], BF16, tag="bcv")
            smp_cm = tc.tile_pool(name="smp", bufs=2,
                                  space=bass.MemorySpace.PSUM)
            smp = smp_cm.__enter__()
            xps_cm = tc.tile_pool(name="xps", bufs=2,
                                  space=bass.MemorySpace.PSUM)
            xps = xps_cm.__enter__()

            def d1_group(cc):
                ps = smp.tile([128, 4, 128], F32, tag="smpsum", name="ps")
                smooth_group("d1", cc, dd, ct["KMD"], ps, "H")
                copyout(s1d[:, 4 * cc:4 * cc + 4, 3:131], ps[:],
                        COPY_ENG["d1"])

            def v1_group(cc):
                ps = smp.tile([128, 4, 128], F32, tag="smpsum", name="ps")
                smooth_group("v1", cc, vn, ct["KBD"], ps, "H")
                copyout(s1[:, 4 * cc:4 * cc + 4, 3:131], ps[:],
                        COPY_ENG["v1"])

            def d2_group(cc):
                g4 = 4 * cc
                xq = xps.tile([128, 4, 128], F32, tag="xq", name="xq")
                smooth_group("d2", cc, s1d, ct["KI"], xq, "W")
                ec = tzp.tile([128, 4, 128], BF16, tag="ec", name="ec")
                nc.scalar.activation(ec[:], xq[:], AF.Exp, scale=-C)
                nc.vector.scalar_tensor_tensor(
                    bcv[:, g4:g4 + 4, :], xq[:], 1.0 / C, ec[:],
                    AL.add, AL.mult)

            with tc.tile_pool(name="cps", bufs=1,
                              space=bass.MemorySpace.PSUM) as cps:

                def curl_group(gi):
                    s0, cnt = GROUPS[gi]
                    pcu = cps.tile([128, 4, 128], F32, tag="pcu", name="pcu")[:, 0:cnt, :]
                    pcv = cps.tile([128, 4, 128], F32, tag="pcv", name="pcv")[:, 0:cnt, :]
                    pcw = cps.tile([128, 4, 128], F32, tag="pcw", name="pcw")[:, 0:cnt, :]
                    nc.tensor.matmul(pcu, ct["IP"],
                                     dw[:, s0 + 1:s0 + 1 + cnt, 0:128],
                                     start=True, stop=False)
                    nc.tensor.matmul(pcu, ct["IN"],
                                     dw[:, s0:s0 + cnt, 0:128],
                                     start=False, stop=False)
                    nc.tensor.matmul(pcu, ct["MDZTN"],
                                     dv[:, s0:s0 + cnt, 0:128],
                                     start=False, stop=True)

                    nc.tensor.matmul(pcv, ct["MDZT"],
                                     du[:, s0:s0 + cnt, 0:128],
                                     start=True, stop=False)
                    nc.tensor.matmul(pcv, ct["IN"],
                                     dw[:, s0:s0 + cnt, 1:129],
                                     start=False, stop=False)
                    nc.tensor.matmul(pcv, ct["IP"],
                                     dw[:, s0:s0 + cnt, 0:128],
                                     start=False, stop=True)

                    nc.tensor.matmul(pcw, ct["IP"],
                                     dv[:, s0:s0 + cnt, 1:129],
                                     start=True, stop=False)
                    nc.tensor.matmul(pcw, ct["IN"],
                                     dv[:, s0:s0 + cnt, 0:128],
                                     start=False, stop=False)
                    nc.tensor.matmul(pcw, ct["IN"],
                                     du[:, s0 + 1:s0 + 1 + cnt, 0:128],
                                     start=False, stop=False)
                    nc.tensor.matmul(pcw, ct["IP"],
                                     du[:, s0:s0 + cnt, 0:128],
                                     start=False, stop=True)

                    squ = sqp.tile([128, 4, 128], BF16, tag="squ", name="squ")[:, 0:cnt, :]
                    sqv = sqp.tile([128, 4, 128], BF16, tag="sqv", name="sqv")[:, 0:cnt, :]
                    sqw = sqp.tile([128, 4, 128], BF16, tag="sqw", name="sqw")[:, 0:cnt, :]
                    nc.scalar.activation(squ, pcu, AF.Square)
                    nc.scalar.activation(sqv, pcv, AF.Square)
                    if gi in SQW_DVE:
                        nc.vector.tensor_mul(sqw, pcw, pcw)
                    else:
                        nc.scalar.activation(sqw, pcw, AF.Square)
                    tsum = sqp.tile([128, 4, 128], BF16, tag="tsum", name="tsum")[:, 0:cnt, :]
                    nc.gpsimd.tensor_add(tsum, squ, sqv)
                    nc.vector.tensor_add(vn[:, s0:s0 + cnt, :], tsum, sqw)

                d1n, d2n, v1n = 0, 0, 0
                for gi in range(len(GROUPS)):
                    curl_group(gi)
                    if gi == 0:
                        nc.vector.tensor_scalar(vn[:, 0:3, :], vn[:, 0:3, :],
                                                mt[:, 0:1], None, AL.mult)
                    if gi >= 1 and d1n < 16:
                        d1_group(d1n)
                        d1n += 1
                    if gi >= 4 and d2n < 16 and d2n <= gi - 4:
                        d2_group(d2n)
                        d2n += 1
                    if gi == 8:
                        nc.scalar.activation(vn[:, 0:36, :], vn[:, 0:36, :],
                                             AF.Sqrt)
                    if gi == 13:
                        nc.scalar.activation(vn[:, 36:52, :], vn[:, 36:52, :],
                                             AF.Sqrt)
                    if gi >= 9 and v1n < 10 and v1n <= gi - 9:
                        v1_group(v1n)
                        v1n += 1
                # tail: finish d2, sqrt rest (ungates remaining v1)
                for cc in range(d2n, 16):
                    d2_group(cc)
                nc.vector.tensor_scalar(vn[:, 67:70, :], vn[:, 67:70, :],
                                        mt[:, 1:2], None, AL.mult)
                nc.scalar.activation(vn[:, 52:70, :], vn[:, 52:70, :],
                                     AF.Sqrt)
            xps_cm.__exit__(None, None, None)

            # ------------- vn W pass + integration -------------
            with tc.tile_pool(name="vps", bufs=3,
                              space=bass.MemorySpace.PSUM) as vps, \
                 tc.tile_pool(name="mps", bufs=2,
                              space=bass.MemorySpace.PSUM) as mps, \
                 tc.tile_pool(name="ivp", bufs=1,
                              space=bass.MemorySpace.PSUM) as ivp:
                ivq = ivp.tile([128, 512], F32, tag="rps", name="ivq")
                nc.vector.memset(ivq[0:65, :], 0.0)
                for cc in range(16):
                    if v1n < 16:
                        v1_group(v1n)
                        v1n += 1
                    g4 = 4 * cc
                    vq = vps.tile([128, 4, 128], F32, tag="vq", name="vq")
                    smooth_group("v2", cc, s1, ct["KI"], vq, "W")
                    mq = mps.tile([128, 4, 128], F32, tag="mq", name="mq")
                    nc.tensor.matmul(mq[:], ct["MCTC"],
                                     bcv[:, g4:g4 + 4, :],
                                     start=True, stop=True)
                    mq_sb = tzp.tile([128, 4, 128], BF16, tag="mqsb",
                                     name="mq_sb")
                    nc.scalar.activation(mq_sb[:], mq[:], AF.Identity,
                                         bias=mt[:, 2:3])
                    pchunk = tzp.tile([128, 4, 128], BF16, tag="pchunk",
                                      name="pchunk")
                    nc.vector.tensor_mul(pchunk[:], mq_sb[:], vq[:])
                    T, r = cc // 3, cc % 3
                    nc.tensor.matmul(ivq[32 * r:32 * r + 1, :], ct["ONESC"],
                                     pchunk[:], start=True, stop=True)
                    if r == 2 or cc == 15:
                        nc.vector.tensor_scalar(iv_sb[0:65, T, :],
                                                ivq[0:65, :],
                                                1.0, 0.0, AL.min, AL.max)
                for T in range(6):
                    rows = min(3, 16 - 3 * T)
                    nc.sync.dma_start(out_t[3 * T:3 * T + rows, :],
                                      iv_sb[0:32 * (rows - 1) + 1:32, T, :])
            smp_cm.__exit__(None, None, None)

    nc.compile()
    return nc


def _pack_consts():
    cm = CONSTS
    ccrl = np.stack([cm["IP"], cm["IN"], cm["MDZT"], cm["MDZTN"]],
                    axis=1).astype(NPBF)
    csm = np.concatenate([cm["KBD"], cm["KMD"], cm["KI"]], axis=1).astype(NPBF)
    ctz = np.concatenate([cm["MCTC"], cm["E127"], cm["ONESC"]],
                         axis=1).astype(NPBF)
    return ccrl, csm, ctz


def host_prepare(d_np, v_np):
    cores = []
    ccrl, csm, ctz = _pack_consts()
    vext = np.zeros((3, 128, 135, 129), np.float32)
    for c in range(8):
        b, hh = c // 2, c % 2
        h0 = 64 * hh
        dpad = np.zeros((128, 70, 128), np.float32)
        lo, hi = h0 - 3, h0 + 67
        src_lo, src_hi = max(lo, 0), min(hi, 128)
        dpad[:, (src_lo - lo):(src_hi - lo), :] = \
            d_np[b, 0, :, src_lo:src_hi, :]
        vext[:] = 0.0
        vext[:, :, 3:131, 0:128] = v_np[b]
        vext[:, :, 131, 0:128] = 2 * v_np[b, :, :, 127, :] - v_np[b, :, :, 126, :]
        vext[:, :, :, 128] = 2 * vext[:, :, :, 127] - vext[:, :, :, 126]
        vin = np.ascontiguousarray(vext[:, :, h0:h0 + 71, :]).astype(NPBF)
        mm = np.zeros((128, 4), np.float32)
        mm[:, 0] = 0.0 if hh == 0 else 1.0
        mm[:, 1] = 0.0 if hh == 1 else 1.0
        mm[127, 2] = 1.0
        m = {
            "d_in": dpad.astype(NPBF), "v_in": vin, "m_in": mm,
            "ccrl_in": ccrl, "csm_in": csm, "ctz_in": ctz,
        }
        cores.append(m)
    return cores


_NC = None


def kernel(d, v):
    global _NC
    d = np.asarray(d, np.float32)
    v = np.asarray(v, np.float32)
    if _NC is None:
        _NC = build_program()
    in_maps = host_prepare(d, v)
    res = run_bass_kernel_spmd(_NC, in_maps, list(range(8)))
    out = np.zeros((4, 1, 128, 128), np.float32)
    for c in range(8):
        b, hh = c // 2, c % 2
        out[b, 0, 64 * hh:64 * hh + 64, :] = \
            res.results[c]["out"].reshape(64, 128)
    return out


# revision 30
# speedup vs baseline: 1.6815x; 1.0222x over previous
"""Trainium2 Bass kernel for DiffVorticeSketchRender (v2, bf16).

Sharding: 8 cores = 4 batches x 2 H-halves (64 rows each + halos).
Device layout: [D=128 partitions, H slices, W free] everywhere, all bf16
(inputs converted on host; PSUM accumulation stays fp32).

- curl + |curl|^2: PSUM matmuls with +/-I and a D-difference band matrix;
  squares on ScalarE, sums on VectorE, sqrt chunks on ScalarE.
- 3D gaussian smooth of vn: 2 matmul passes using the symmetric-tap trick:
  pa_j = win_j + win_{6-j} on VectorE/GpSimd, then 4 accumulated matmuls
  (D-band fused into the H pass matrices).
- d path: pass1 fuses M_x = SUF@flip@BD (D-conv + depth flip + cumsum in a
  single 128x128 matrix) with the H taps; pass2 applies the W taps, giving
  the optical depth x directly in PSUM.
- transmittance/integration: exp on ScalarE straight from PSUM,
  bc=(x+1/C)*exp(-Cx) via scalar_tensor_tensor, trapezoid coefficients via
  one matmul with C*MC, front term folded in as a per-partition +e127
  Identity-bias on the PSUM->SBUF copy, ones-matmul reductions targeting
  partition bases {0,32,64} of a shared PSUM tile, fused clip+copy via one
  tensor_scalar(min,max) per 3 chunks, stride-32 partition DMA gather.

Hardware rules learned (enforced by walrus, not TimelineSim): DVE/ACT read
at most ONE PSUM operand per instruction; ACT/DVE accesses must start at
partition 0 (matmul PSUM outs may use bases {0,32,64}); GpSimd has no PSUM
port; DMA never touches PSUM; Sqrt and Exp live in different ACT table sets
(1283ns per switch).
"""

import numpy as np
import ml_dtypes

import concourse.bacc as bacc
import concourse.bass as bass
import concourse.mybir as mybir
import concourse.tile as tile
from concourse.bass_utils import run_bass_kernel_spmd

F32 = mybir.dt.float32
BF16 = mybir.dt.bfloat16
AL = mybir.AluOpType
AF = mybir.ActivationFunctionType
NPBF = ml_dtypes.bfloat16

KHS, SIGMA, C = 3, 1.6, 20.0

# ---- tuning knobs ----
SYM = {"d1": True, "v1": True, "v2": True, "d2": True}
# engine for each of the 3 sym pre-adds per group: "v"=vector, "p"=gpsimd
PREADD_ENG = {"d1": "vvv", "v1": "vvp", "v2": "vvp", "d2": "vvp"}
COPY_ENG = {"d1": "v", "v1": "s"}  # psum->sbuf copy engine per pass
SQW_DVE = set()


def _gauss1d():
    size = 2 * KHS + 1
    g = np.arange(size, dtype=np.float64) - (size - 1) / 2.0
    g = np.exp(-((g / SIGMA) ** 2) / 2.0) / (SIGMA * np.sqrt(2.0 * np.pi))
    return g / g.sum()  # per-axis normalized (3d product kernel sums to 1)


def _const_mats():
    gn = _gauss1d()
    eye = np.eye(128, dtype=np.float64)

    # BD: D-axis 'same' zero-padded conv: out[dp] = sum_k gn[k] * in[dp+k-3]
    bd = np.zeros((128, 128), np.float64)
    for dp in range(128):
        for k in range(7):
            d = dp + k - 3
            if 0 <= d < 128:
                bd[dp, d] = gn[k]

    # M_x = SUF @ flip @ BD : x_flip = cumsum(flip(BD @ v))
    mx = np.cumsum(bd[::-1, :], axis=0)

    # trapezoid coefficient matrix MC (consumes t stored in flipped order):
    # c[m] coefficient of vns[m]; c = MC @ t (+1 at m=127 added separately)
    mc = np.zeros((128, 128), np.float64)
    for m in range(1, 127):
        mc[m, 126 - m] += 0.5
        mc[m, 128 - m] -= 0.5
    mc[0, 126] += 0.5
    mc[0, 127] -= 0.5
    mc[127, 0] -= 0.5
    mc[127, 1] -= 0.5

    mdz = np.zeros((128, 128), np.float64)
    for d in range(127):
        mdz[d, d] = -1.0
        mdz[d, d + 1] = 1.0
    mdz[127, 126] = -1.0
    mdz[127, 127] = 1.0

    kbd = np.stack([(gn[j] * bd).T for j in range(4)], axis=1)   # [128,4,128]
    kmd = np.stack([(gn[j] * mx).T for j in range(4)], axis=1)   # [128,4,128]
    ki = np.stack([gn[j] * eye for j in range(4)], axis=1)       # [128,4,128]

    e127 = np.zeros((128, 1), np.float64)
    e127[127, 0] = 1.0

    out = {
        "KBD": kbd, "KMD": kmd, "KI": ki,
        "MDZT": mdz.T.copy(), "MDZTN": (-mdz.T).copy(),
        "IP": eye, "IN": -eye,
        "MCTC": (C * mc).T.copy(),
        "E127": e127, "ONESC": np.ones((128, 1), np.float64),
    }
    return {k: v.astype(NPBF) for k, v in out.items()}


def _curl_groups():
    gs = []
    s0 = 0
    while s0 < 70:
        cnt = min(4, 70 - s0)
        gs.append((s0, cnt))
        s0 += cnt
    return gs


CONSTS = _const_mats()


def build_program():
    nc = bacc.Bacc("TRN2", target_bir_lowering=False, debug=False)

    d_in = nc.dram_tensor("d_in", [128, 70, 128], BF16, kind="ExternalInput")
    v_in = nc.dram_tensor("v_in", [3, 128, 71, 129], BF16, kind="ExternalInput")
    m_in = nc.dram_tensor("m_in", [128, 4], F32, kind="ExternalInput")
    ccrl_in = nc.dram_tensor("ccrl_in", [128, 4, 128], BF16, kind="ExternalInput")
    csm_in = nc.dram_tensor("csm_in", [128, 12, 128], BF16, kind="ExternalInput")
    ctz_in = nc.dram_tensor("ctz_in", [128, 130], BF16, kind="ExternalInput")
    out_t = nc.dram_tensor("out", [16, 512], F32, kind="ExternalOutput")

    with tile.TileContext(nc) as tc:
        with tc.tile_pool(name="const", bufs=1) as cp, \
             tc.tile_pool(name="vol", bufs=1) as vol, \
             tc.tile_pool(name="sq", bufs=2) as sqp, \
             tc.tile_pool(name="pa", bufs=2) as pap, \
             tc.tile_pool(name="tz", bufs=4) as tzp:
            ccrl = cp.tile([128, 4, 128], BF16, tag="ccrl")
            csm = cp.tile([128, 12, 128], BF16, tag="csm")
            ctz = cp.tile([128, 130], BF16, tag="ctz")
            mt = cp.tile([128, 4], F32, tag="mt")

            vt = vol.tile([128, 3, 71, 129], BF16, tag="vt")
            dd = vol.tile([128, 70, 128], BF16, tag="dd")
            vn = vol.tile([128, 70, 128], BF16, tag="vn")
            s1 = vol.tile([128, 64, 134], BF16, tag="s1")
            s1d = vol.tile([128, 64, 134], BF16, tag="s1d")
            iv_sb = vol.tile([65, 6, 512], F32, tag="ivsb")

            # DMA order tuned so curl group 0 can start ~1.5us in and the
            # feed stays ahead of curl groups
            nc.sync.dma_start(ccrl[:], ccrl_in[:])
            for a, b in ((0, 4), (4, 8)):
                for c in (2, 1, 0):
                    nc.sync.dma_start(vt[:, c, a:b, :], v_in[c, :, a:b, :])
            nc.sync.dma_start(csm[:], csm_in[:])
            nc.sync.dma_start(dd[:, 0:10, :], d_in[:, 0:10, :])
            for c in (2, 1, 0):
                nc.sync.dma_start(vt[:, c, 8:13, :], v_in[c, :, 8:13, :])
            nc.sync.dma_start(dd[:, 10:40, :], d_in[:, 10:40, :])
            for c in range(3):
                nc.sync.dma_start(vt[:, c, 13:25, :], v_in[c, :, 13:25, :])
            nc.sync.dma_start(dd[:, 40:70, :], d_in[:, 40:70, :])
            for a, b in ((25, 37), (37, 49), (49, 61), (61, 71)):
                for c in range(3):
                    nc.sync.dma_start(vt[:, c, a:b, :], v_in[c, :, a:b, :])
            nc.sync.dma_start(ctz[:], ctz_in[:])
            nc.sync.dma_start(mt[:], m_in[:])
            for t in (s1, s1d):
                nc.gpsimd.memset(t[:, :, 0:3], 0.0)
                nc.gpsimd.memset(t[:, :, 131:134], 0.0)

            ct = {
                "IP": ccrl[:, 0, :], "IN": ccrl[:, 1, :],
                "MDZT": ccrl[:, 2, :], "MDZTN": ccrl[:, 3, :],
                "KBD": 0, "KMD": 4, "KI": 8,
                "MCTC": ctz[:, 0:128], "E127": ctz[:, 128:129],
                "ONESC": ctz[:, 129:130],
            }
            du, dv, dw = vt[:, 0], vt[:, 1], vt[:, 2]

            def preadd(dst, a, b, eng):
                if eng == "v":
                    nc.vector.tensor_add(dst, a, b)
                else:
                    nc.gpsimd.tensor_add(dst, a, b)

            def smooth_group(name, cc, src, kbase, psum_t, axis):
                """4-output-slice group cc: 7-tap conv via 4 matmuls (sym)."""
                kmat = lambda j: csm[:, kbase + j, :]
                g4 = 4 * cc
                if axis == "H":
                    win = lambda j: src[:, g4 + j:g4 + j + 4, :]
                else:
                    win = lambda j: src[:, g4:g4 + 4, j:j + 128]
                if SYM[name]:
                    engs = PREADD_ENG[name]
                    pas = []
                    for j in range(3):
                        pa = pap.tile([128, 4, 128], BF16, tag=f"pa{name}{j}",
                                      name="pa")
                        preadd(pa[:], win(j), win(6 - j), engs[j])
                        pas.append(pa)
                    for j in range(3):
                        nc.tensor.matmul(psum_t[:], kmat(j), pas[j][:],
                                         start=(j == 0), stop=False)
                    nc.tensor.matmul(psum_t[:], kmat(3), win(3),
                                     start=False, stop=True)
                else:
                    for j in range(7):
                        jj = min(j, 6 - j)
                        nc.tensor.matmul(psum_t[:], kmat(jj), win(j),
                                         start=(j == 0), stop=(j == 6))

            def copyout(dst, src, eng):
                if eng == "s":
                    nc.scalar.copy(dst, src)
                else:
                    nc.vector.tensor_copy(dst, src)

            # ------------- interleaved schedule -------------
            GROUPS = _curl_groups()
            bcv = vol.tile([128, 64, 128], BF16, tag="bcv")
            smp_cm = tc.tile_pool(name="smp", bufs=2,
                                  space=bass.MemorySpace.PSUM)
            smp = smp_cm.__enter__()
            xps_cm = tc.tile_pool(name="xps", bufs=2,
                                  space=bass.MemorySpace.PSUM)
            xps = xps_cm.__enter__()

            def d1_group(cc):
                ps = smp.tile([128, 4, 128], F32, tag="smpsum", name="ps")
                smooth_group("d1", cc, dd, ct["KMD"], ps, "H")
                copyout(s1d[:, 4 * cc:4 * cc + 4, 3:131], ps[:],
                        COPY_ENG["d1"])

            def v1_group(cc):
                ps = smp.tile([128, 4, 128], F32, tag="smpsum", name="ps")
                smooth_group("v1", cc, vn, ct["KBD"], ps, "H")
                copyout(s1[:, 4 * cc:4 * cc + 4, 3:131], ps[:],
                        COPY_ENG["v1"])

            def d2_group(cc):
                g4 = 4 * cc
                xq = xps.tile([128, 4, 128], F32, tag="xq", name="xq")
                smooth_group("d2", cc, s1d, ct["KI"], xq, "W")
                ec = tzp.tile([128, 4, 128], BF16, tag="ec", name="ec")
                nc.scalar.activation(ec[:], xq[:], AF.Exp, scale=-C)
                nc.vector.scalar_tensor_tensor(
                    bcv[:, g4:g4 + 4, :], xq[:], 1.0 / C, ec[:],
                    AL.add, AL.mult)

            with tc.tile_pool(name="cps", bufs=1,
                              space=bass.MemorySpace.PSUM) as cps:

                def curl_group(gi):
                    s0, cnt = GROUPS[gi]
                    pcu = cps.tile([128, 4, 128], F32, tag="pcu", name="pcu")[:, 0:cnt, :]
                    pcv = cps.tile([128, 4, 128], F32, tag="pcv", name="pcv")[:, 0:cnt, :]
                    pcw = cps.tile([128, 4, 128], F32, tag="pcw", name="pcw")[:, 0:cnt, :]
                    nc.tensor.matmul(pcu, ct["IP"],
                                     dw[:, s0 + 1:s0 + 1 + cnt, 0:128],
                                     start=True, stop=False)
                    nc.tensor.matmul(pcu, ct["IN"],
                                     dw[:, s0:s0 + cnt, 0:128],
                                     start=False, stop=False)
                    nc.tensor.matmul(pcu, ct["MDZTN"],
                                     dv[:, s0:s0 + cnt, 0:128],
                                     start=False, stop=True)

                    nc.tensor.matmul(pcv, ct["MDZT"],
                                     du[:, s0:s0 + cnt, 0:128],
                                     start=True, stop=False)
                    nc.tensor.matmul(pcv, ct["IN"],
                                     dw[:, s0:s0 + cnt, 1:129],
                                     start=False, stop=False)
                    nc.tensor.matmul(pcv, ct["IP"],
                                     dw[:, s0:s0 + cnt, 0:128],
                                     start=False, stop=True)

                    nc.tensor.matmul(pcw, ct["IP"],
                                     dv[:, s0:s0 + cnt, 1:129],
                                     start=True, stop=False)
                    nc.tensor.matmul(pcw, ct["IN"],
                                     dv[:, s0:s0 + cnt, 0:128],
                                     start=False, stop=False)
                    nc.tensor.matmul(pcw, ct["IN"],
                                     du[:, s0 + 1:s0 + 1 + cnt, 0:128],
                                     start=False, stop=False)
                    nc.tensor.matmul(pcw, ct["IP"],
                                     du[:, s0:s0 + cnt, 0:128],
                                     start=False, stop=True)

                    squ = sqp.tile([128, 4, 128], BF16, tag="squ", name="squ")[:, 0:cnt, :]
                    sqv = sqp.tile([128, 4, 128], BF16, tag="sqv", name="sqv")[:, 0:cnt, :]
                    sqw = sqp.tile([128, 4, 128], BF16, tag="sqw", name="sqw")[:, 0:cnt, :]
                    nc.scalar.activation(squ, pcu, AF.Square)
                    nc.scalar.activation(sqv, pcv, AF.Square)
                    if gi in SQW_DVE:
                        nc.vector.tensor_mul(sqw, pcw, pcw)
                    else:
                        nc.scalar.activation(sqw, pcw, AF.Square)
                    tsum = sqp.tile([128, 4, 128], BF16, tag="tsum", name="tsum")[:, 0:cnt, :]
                    nc.gpsimd.tensor_add(tsum, squ, sqv)
                    nc.vector.tensor_add(vn[:, s0:s0 + cnt, :], tsum, sqw)

                d1n, d2n, v1n = 0, 0, 0
                for gi in range(len(GROUPS)):
                    curl_group(gi)
                    if gi == 0:
                        nc.vector.tensor_scalar(vn[:, 0:3, :], vn[:, 0:3, :],
                                                mt[:, 0:1], None, AL.mult)
                    if gi >= 1 and d1n < 16:
                        d1_group(d1n)
                        d1n += 1
                    if gi >= 4 and d2n < 16 and d2n <= gi - 4:
                        d2_group(d2n)
                        d2n += 1
                    if gi == 8:
                        nc.scalar.activation(vn[:, 0:36, :], vn[:, 0:36, :],
                                             AF.Sqrt)
                    if gi == 13:
                        nc.scalar.activation(vn[:, 36:52, :], vn[:, 36:52, :],
                                             AF.Sqrt)
                    if gi >= 9 and v1n < 10 and v1n <= gi - 9:
                        v1_group(v1n)
                        v1n += 1
                # tail: finish d2, sqrt rest (ungates remaining v1)
                for cc in range(d2n, 16):
                    d2_group(cc)
                nc.vector.tensor_scalar(vn[:, 67:70, :], vn[:, 67:70, :],
                                        mt[:, 1:2], None, AL.mult)
                nc.scalar.activation(vn[:, 52:70, :], vn[:, 52:70, :],
                                     AF.Sqrt)
            xps_cm.__exit__(None, None, None)

            # ------------- vn W pass + integration -------------
            with tc.tile_pool(name="vps", bufs=3,
                              space=bass.MemorySpace.PSUM) as vps, \
                 tc.tile_pool(name="mps", bufs=2,
                              space=bass.MemorySpace.PSUM) as mps, \
                 tc.tile_pool(name="ivp", bufs=1,
                              space=bass.MemorySpace.PSUM) as ivp:
                ivq = ivp.tile([128, 512], F32, tag="rps", name="ivq")
                nc.vector.memset(ivq[0:65, :], 0.0)
                for cc in range(16):
                    if v1n < 16:
                        v1_group(v1n)
                        v1n += 1
                    g4 = 4 * cc
                    vq = vps.tile([128, 4, 128], F32, tag="vq", name="vq")
                    smooth_group("v2", cc, s1, ct["KI"], vq, "W")
                    mq = mps.tile([128, 4, 128], F32, tag="mq", name="mq")
                    nc.tensor.matmul(mq[:], ct["MCTC"],
                                     bcv[:, g4:g4 + 4, :],
                                     start=True, stop=True)
                    mq_sb = tzp.tile([128, 4, 128], BF16, tag="mqsb",
                                     name="mq_sb")
                    nc.scalar.activation(mq_sb[:], mq[:], AF.Identity,
                                         bias=mt[:, 2:3])
                    pchunk = tzp.tile([128, 4, 128], BF16, tag="pchunk",
                                      name="pchunk")
                    nc.vector.tensor_mul(pchunk[:], mq_sb[:], vq[:])
                    T, r = cc // 3, cc % 3
                    nc.tensor.matmul(ivq[32 * r:32 * r + 1, :], ct["ONESC"],
                                     pchunk[:], start=True, stop=True)
                    if r == 2 or cc == 15:
                        nc.vector.tensor_scalar(iv_sb[0:65, T, :],
                                                ivq[0:65, :],
                                                1.0, 0.0, AL.min, AL.max)
                for T in range(6):
                    rows = min(3, 16 - 3 * T)
                    nc.sync.dma_start(out_t[3 * T:3 * T + rows, :],
                                      iv_sb[0:32 * (rows - 1) + 1:32, T, :])
            smp_cm.__exit__(None, None, None)

    nc.compile()
    return nc


def _pack_consts():
    cm = CONSTS
    ccrl = np.stack([cm["IP"], cm["IN"], cm["MDZT"], cm["MDZTN"]],
                    axis=1).astype(NPBF)
    csm = np.concatenate([cm["KBD"], cm["KMD"], cm["KI"]], axis=1).astype(NPBF)
    ctz = np.concatenate([cm["MCTC"], cm["E127"], cm["ONESC"]],
                         axis=1).astype(NPBF)
    return ccrl, csm, ctz


def host_prepare(d_np, v_np):
    cores = []
    ccrl, csm, ctz = _pack_consts()
    vext = np.zeros((3, 128, 135, 129), np.float32)
    for c in range(8):
        b, hh = c // 2, c % 2
        h0 = 64 * hh
        dpad = np.zeros((128, 70, 128), np.float32)
        lo, hi = h0 - 3, h0 + 67
        src_lo, src_hi = max(lo, 0), min(hi, 128)
        dpad[:, (src_lo - lo):(src_hi - lo), :] = \
            d_np[b, 0, :, src_lo:src_hi, :]
        vext[:] = 0.0
        vext[:, :, 3:131, 0:128] = v_np[b]
        vext[:, :, 131, 0:128] = 2 * v_np[b, :, :, 127, :] - v_np[b, :, :, 126, :]
        vext[:, :, :, 128] = 2 * vext[:, :, :, 127] - vext[:, :, :, 126]
        vin = np.ascontiguousarray(vext[:, :, h0:h0 + 71, :]).astype(NPBF)
        mm = np.zeros((128, 4), np.float32)
        mm[:, 0] = 0.0 if hh == 0 else 1.0
        mm[:, 1] = 0.0 if hh == 1 else 1.0
        mm[127, 2] = 1.0
        m = {
            "d_in": dpad.astype(NPBF), "v_in": vin, "m_in": mm,
            "ccrl_in": ccrl, "csm_in": csm, "ctz_in": ctz,
        }
        cores.append(m)
    return cores


_NC = None


def kernel(d, v):
    global _NC
    d = np.asarray(d, np.float32)
    v = np.asarray(v, np.float32)
    if _NC is None:
        _NC = build_program()
    in_maps = host_prepare(d, v)
    res = run_bass_kernel_spmd(_NC, in_maps, list(range(8)))
    out = np.zeros((4, 1, 128, 128), np.float32)
    for c in range(8):
        b, hh = c // 2, c % 2
        out[b, 0, 64 * hh:64 * hh + 64, :] = \
            res.results[c]["out"].reshape(64, 128)
    return out


# revision 33
# speedup vs baseline: 1.8065x; 1.0744x over previous
"""Trainium2 Bass kernel for DiffVorticeSketchRender (v2, bf16).

Sharding: 8 cores = 4 batches x 2 H-halves (64 rows each + halos).
Device layout: [D=128 partitions, H slices, W free] everywhere, all bf16
(inputs converted on host; PSUM accumulation stays fp32).

- curl + |curl|^2: PSUM matmuls with +/-I and a D-difference band matrix;
  squares on ScalarE, sums on VectorE, sqrt chunks on ScalarE.
- 3D gaussian smooth of vn: 2 matmul passes using the symmetric-tap trick:
  pa_j = win_j + win_{6-j} on VectorE/GpSimd, then 4 accumulated matmuls
  (D-band fused into the H pass matrices).
- d path: pass1 fuses M_x = SUF@flip@BD (D-conv + depth flip + cumsum in a
  single 128x128 matrix) with the H taps; pass2 applies the W taps, giving
  the optical depth x directly in PSUM.
- transmittance/integration: exp on ScalarE straight from PSUM,
  bc=(x+1/C)*exp(-Cx) via scalar_tensor_tensor, trapezoid coefficients via
  one matmul with C*MC, front term folded in as a per-partition +e127
  Identity-bias on the PSUM->SBUF copy, ones-matmul reductions targeting
  partition bases {0,32,64} of a shared PSUM tile, fused clip+copy via one
  tensor_scalar(min,max) per 3 chunks, stride-32 partition DMA gather.

Hardware rules learned (enforced by walrus, not TimelineSim): DVE/ACT read
at most ONE PSUM operand per instruction; ACT/DVE accesses must start at
partition 0 (matmul PSUM outs may use bases {0,32,64}); GpSimd has no PSUM
port; DMA never touches PSUM; Sqrt and Exp live in different ACT table sets
(1283ns per switch).
"""

import numpy as np
import ml_dtypes

import concourse.bacc as bacc
import concourse.bass as bass
import concourse.mybir as mybir
import concourse.tile as tile
from concourse.ap import AP
from concourse.bass_utils import run_bass_kernel_spmd

F32 = mybir.dt.float32
BF16 = mybir.dt.bfloat16
AL = mybir.AluOpType
AF = mybir.ActivationFunctionType
NPBF = ml_dtypes.bfloat16
FP8 = mybir.dt.float8e4
NPF8 = ml_dtypes.float8_e4m3

KHS, SIGMA, C = 3, 1.6, 20.0

# ---- tuning knobs ----
SYM = {"d1": False, "v1": True, "v2": True, "d2": True}
# engine for each of the 3 sym pre-adds per group: "v"=vector, "p"=gpsimd
PREADD_ENG = {"d1": "vvv", "v1": "vvv", "v2": "vvp", "d2": "vvp"}
COPY_ENG = {"d1": "v", "v1": "s"}  # psum->sbuf copy engine per pass
SQW_DVE = set()


def _gauss1d():
    size = 2 * KHS + 1
    g = np.arange(size, dtype=np.float64) - (size - 1) / 2.0
    g = np.exp(-((g / SIGMA) ** 2) / 2.0) / (SIGMA * np.sqrt(2.0 * np.pi))
    return g / g.sum()  # per-axis normalized (3d product kernel sums to 1)


def _const_mats():
    gn = _gauss1d()
    eye = np.eye(128, dtype=np.float64)

    # BD: D-axis 'same' zero-padded conv: out[dp] = sum_k gn[k] * in[dp+k-3]
    bd = np.zeros((128, 128), np.float64)
    for dp in range(128):
        for k in range(7):
            d = dp + k - 3
            if 0 <= d < 128:
                bd[dp, d] = gn[k]

    # M_x = SUF @ flip @ BD : x_flip = cumsum(flip(BD @ v))
    mx = np.cumsum(bd[::-1, :], axis=0)

    # trapezoid coefficient matrix MC (consumes t stored in flipped order):
    # c[m] coefficient of vns[m]; c = MC @ t (+1 at m=127 added separately)
    mc = np.zeros((128, 128), np.float64)
    for m in range(1, 127):
        mc[m, 126 - m] += 0.5
        mc[m, 128 - m] -= 0.5
    mc[0, 126] += 0.5
    mc[0, 127] -= 0.5
    mc[127, 0] -= 0.5
    mc[127, 1] -= 0.5

    mdz = np.zeros((128, 128), np.float64)
    for d in range(127):
        mdz[d, d] = -1.0
        mdz[d, d + 1] = 1.0
    mdz[127, 126] = -1.0
    mdz[127, 127] = 1.0

    kbd = np.stack([(gn[j] * bd).T for j in range(4)], axis=1)   # [128,4,128]
    kmd = np.stack([(gn[j] * mx).T for j in range(4)], axis=1)   # [128,4,128]
    ki = np.stack([gn[j] * eye for j in range(4)], axis=1)       # [128,4,128]

    e127 = np.zeros((128, 1), np.float64)
    e127[127, 0] = 1.0

    out = {
        "KBD": kbd, "KMD": kmd, "KI": ki,
        "MCTC": (C * mc).T.copy(),
        "E127": e127, "ONESC": np.ones((128, 1), np.float64),
    }
    out = {k: v.astype(NPBF) for k, v in out.items()}
    # fp8 DoubleRow curl matrix pairs: A=[-I|+I], B=[+I|-I], C=[-mdz.T|0],
    # D=[+mdz.T|0]
    z = np.zeros((128, 128))
    out["C8"] = np.stack([
        np.stack([-eye, eye], axis=1),
        np.stack([eye, -eye], axis=1),
        np.stack([-mdz.T, z], axis=1),
        np.stack([mdz.T, z], axis=1),
    ], axis=1).astype(NPF8)          # [128, 4, 2, 128]
    return out


def _curl_groups():
    gs = []
    s0 = 0
    while s0 < 70:
        cnt = min(4, 70 - s0)
        gs.append((s0, cnt))
        s0 += cnt
    return gs


CONSTS = _const_mats()


def build_program():
    nc = bacc.Bacc("TRN2", target_bir_lowering=False, debug=False)

    d_in = nc.dram_tensor("d_in", [128, 70, 128], BF16, kind="ExternalInput")
    v_in = nc.dram_tensor("v_in", [3, 128, 71, 129], FP8, kind="ExternalInput")
    m_in = nc.dram_tensor("m_in", [128, 4], F32, kind="ExternalInput")
    c8_in = nc.dram_tensor("c8_in", [128, 4, 2, 128], FP8, kind="ExternalInput")
    csm_in = nc.dram_tensor("csm_in", [128, 12, 128], BF16, kind="ExternalInput")
    ctz_in = nc.dram_tensor("ctz_in", [128, 130], BF16, kind="ExternalInput")
    out_t = nc.dram_tensor("out", [16, 512], F32, kind="ExternalOutput")

    with tile.TileContext(nc) as tc:
        with tc.tile_pool(name="const", bufs=1) as cp, \
             tc.tile_pool(name="vol", bufs=1) as vol, \
             tc.tile_pool(name="sq", bufs=2) as sqp, \
             tc.tile_pool(name="pa", bufs=2) as pap, \
             tc.tile_pool(name="tz", bufs=4) as tzp:
            c8 = cp.tile([128, 4, 2, 128], FP8, tag="c8")
            csm = cp.tile([128, 12, 128], BF16, tag="csm")
            ctz = cp.tile([128, 130], BF16, tag="ctz")
            mt = cp.tile([128, 4], F32, tag="mt")

            vt = vol.tile([128, 3, 71, 129], FP8, tag="vt")
            dd = vol.tile([128, 70, 128], BF16, tag="dd")
            vn = vol.tile([128, 70, 128], BF16, tag="vn")
            s1 = vol.tile([128, 64, 134], BF16, tag="s1")
            s1d = vol.tile([128, 64, 134], BF16, tag="s1d")
            iv_sb = vol.tile([65, 6, 512], F32, tag="ivsb")

            # DMA order tuned so curl group 0 can start ~1.5us in and the
            # feed stays ahead of curl groups
            nc.sync.dma_start(c8[:], c8_in[:])
            for a, b in ((0, 4), (4, 8)):
                for c in (2, 1, 0):
                    nc.sync.dma_start(vt[:, c, a:b, :], v_in[c, :, a:b, :])
            nc.sync.dma_start(csm[:], csm_in[:])
            nc.sync.dma_start(dd[:, 0:10, :], d_in[:, 0:10, :])
            for c in (2, 1, 0):
                nc.sync.dma_start(vt[:, c, 8:13, :], v_in[c, :, 8:13, :])
            nc.sync.dma_start(dd[:, 10:40, :], d_in[:, 10:40, :])
            for c in range(3):
                nc.sync.dma_start(vt[:, c, 13:25, :], v_in[c, :, 13:25, :])
            nc.sync.dma_start(dd[:, 40:70, :], d_in[:, 40:70, :])
            for a, b in ((25, 37), (37, 49), (49, 61), (61, 71)):
                for c in range(3):
                    nc.sync.dma_start(vt[:, c, a:b, :], v_in[c, :, a:b, :])
            nc.sync.dma_start(ctz[:], ctz_in[:])
            nc.sync.dma_start(mt[:], m_in[:])
            for t in (s1, s1d):
                nc.gpsimd.memset(t[:, :, 0:3], 0.0)
                nc.gpsimd.memset(t[:, :, 131:134], 0.0)

            ct = {
                "KBD": 0, "KMD": 4, "KI": 8,
                "MCTC": ctz[:, 0:128], "E127": ctz[:, 128:129],
                "ONESC": ctz[:, 129:130],
            }
            DU, DV, DW = 0, 1, 2

            def vpair(chan, s0, cnt, axis):
                """rhs AP [128, 2, cnt, 128]: DoubleRow pair of shifted
                windows of v channel chan (pair stride: 1 H slice or 1 W)."""
                base = vt[:, chan, s0:s0 + cnt, 0:128]
                st = 129 if axis == "H" else 1
                return AP(base.tensor, base.offset,
                          [list(base.ap[0]), [st, 2], [129, cnt], [1, 128]])

            def preadd(dst, a, b, eng):
                if eng == "v":
                    nc.vector.tensor_add(dst, a, b)
                else:
                    nc.gpsimd.tensor_add(dst, a, b)

            def smooth_group(name, cc, src, kbase, psum_t, axis):
                """4-output-slice group cc: 7-tap conv via 4 matmuls (sym)."""
                kmat = lambda j: csm[:, kbase + j, :]
                g4 = 4 * cc
                if axis == "H":
                    win = lambda j: src[:, g4 + j:g4 + j + 4, :]
                else:
                    win = lambda j: src[:, g4:g4 + 4, j:j + 128]
                if SYM[name]:
                    engs = PREADD_ENG[name]
                    pas = []
                    for j in range(3):
                        pa = pap.tile([128, 4, 128], BF16, tag=f"pa{name}{j}",
                                      name="pa")
                        preadd(pa[:], win(j), win(6 - j), engs[j])
                        pas.append(pa)
                    for j in range(3):
                        nc.tensor.matmul(psum_t[:], kmat(j), pas[j][:],
                                         start=(j == 0), stop=False)
                    nc.tensor.matmul(psum_t[:], kmat(3), win(3),
                                     start=False, stop=True)
                else:
                    for j in range(7):
                        jj = min(j, 6 - j)
                        nc.tensor.matmul(psum_t[:], kmat(jj), win(j),
                                         start=(j == 0), stop=(j == 6))

            def copyout(dst, src, eng):
                if eng == "s":
                    nc.scalar.copy(dst, src)
                else:
                    nc.vector.tensor_copy(dst, src)

            # ------------- interleaved schedule -------------
            GROUPS = _curl_groups()
            bcv = vol.tile([128, 64, 128], BF16, tag="bcv")
            smp_cm = tc.tile_pool(name="smp", bufs=2,
                                  space=bass.MemorySpace.PSUM)
            smp = smp_cm.__enter__()
            xps_cm = tc.tile_pool(name="xps", bufs=2,
                                  space=bass.MemorySpace.PSUM)
            xps = xps_cm.__enter__()

            def d1_group(cc):
                ps = smp.tile([128, 4, 128], F32, tag="smpsum", name="ps")
                smooth_group("d1", cc, dd, ct["KMD"], ps, "H")
                copyout(s1d[:, 4 * cc:4 * cc + 4, 3:131], ps[:],
                        COPY_ENG["d1"])

            def v1_group(cc):
                ps = smp.tile([128, 4, 128], F32, tag="smpsum", name="ps")
                smooth_group("v1", cc, vn, ct["KBD"], ps, "H")
                copyout(s1[:, 4 * cc:4 * cc + 4, 3:131], ps[:],
                        COPY_ENG["v1"])

            def d2_group(cc):
                g4 = 4 * cc
                xq = xps.tile([128, 4, 128], F32, tag="xq", name="xq")
                smooth_group("d2", cc, s1d, ct["KI"], xq, "W")
                ec = tzp.tile([128, 4, 128], BF16, tag="ec", name="ec")
                nc.scalar.activation(ec[:], xq[:], AF.Exp, scale=-C)
                nc.vector.scalar_tensor_tensor(
                    bcv[:, g4:g4 + 4, :], xq[:], 1.0 / C, ec[:],
                    AL.add, AL.mult)

            with tc.tile_pool(name="cps", bufs=1,
                              space=bass.MemorySpace.PSUM) as cps:

                def curl_group(gi):
                    s0, cnt = GROUPS[gi]
                    pcu = cps.tile([128, 4, 128], F32, tag="pcu", name="pcu")[:, 0:cnt, :]
                    pcv = cps.tile([128, 4, 128], F32, tag="pcv", name="pcv")[:, 0:cnt, :]
                    pcw = cps.tile([128, 4, 128], F32, tag="pcw", name="pcw")[:, 0:cnt, :]
                    DR = mybir.MatmulPerfMode.DoubleRow
                    # cu = (dw[h+1]-dw[h]) - mdz@dv
                    nc.tensor.matmul(pcu, c8[:, 0, :, :], vpair(DW, s0, cnt, "H"),
                                     start=True, stop=False, perf_mode=DR)
                    nc.tensor.matmul(pcu, c8[:, 2, :, :], vpair(DV, s0, cnt, "H"),
                                     start=False, stop=True, perf_mode=DR)
                    # cv = mdz@du - (dw[x+1]-dw[x])
                    nc.tensor.matmul(pcv, c8[:, 1, :, :], vpair(DW, s0, cnt, "W"),
                                     start=True, stop=False, perf_mode=DR)
                    nc.tensor.matmul(pcv, c8[:, 3, :, :], vpair(DU, s0, cnt, "H"),
                                     start=False, stop=True, perf_mode=DR)
                    # cw = (dv[x+1]-dv[x]) - (du[h+1]-du[h])
                    nc.tensor.matmul(pcw, c8[:, 0, :, :], vpair(DV, s0, cnt, "W"),
                                     start=True, stop=False, perf_mode=DR)
                    nc.tensor.matmul(pcw, c8[:, 1, :, :], vpair(DU, s0, cnt, "H"),
                                     start=False, stop=True, perf_mode=DR)

                    squ = sqp.tile([128, 4, 128], BF16, tag="squ", name="squ")[:, 0:cnt, :]
                    sqv = sqp.tile([128, 4, 128], BF16, tag="sqv", name="sqv")[:, 0:cnt, :]
                    sqw = sqp.tile([128, 4, 128], BF16, tag="sqw", name="sqw")[:, 0:cnt, :]
                    nc.scalar.activation(squ, pcu, AF.Square)
                    nc.scalar.activation(sqv, pcv, AF.Square)
                    if gi in SQW_DVE:
                        nc.vector.tensor_mul(sqw, pcw, pcw)
                    else:
                        nc.scalar.activation(sqw, pcw, AF.Square)
                    tsum = sqp.tile([128, 4, 128], BF16, tag="tsum", name="tsum")[:, 0:cnt, :]
                    nc.gpsimd.tensor_add(tsum, squ, sqv)
                    nc.vector.tensor_add(vn[:, s0:s0 + cnt, :], tsum, sqw)

                d1n, d2n, v1n = 0, 0, 0
                for gi in range(len(GROUPS)):
                    curl_group(gi)
                    if gi == 0:
                        nc.vector.tensor_scalar(vn[:, 0:3, :], vn[:, 0:3, :],
                                                mt[:, 0:1], None, AL.mult)
                    if gi >= 1 and d1n < 16:
                        d1_group(d1n)
                        d1n += 1
                    if gi >= 4 and d2n < 16 and d2n <= gi - 4:
                        d2_group(d2n)
                        d2n += 1
                    if gi == 8:
                        nc.scalar.activation(vn[:, 0:36, :], vn[:, 0:36, :],
                                             AF.Sqrt)
                    if gi == 13:
                        nc.scalar.activation(vn[:, 36:52, :], vn[:, 36:52, :],
                                             AF.Sqrt)
                    if gi >= 9 and v1n < 10 and v1n <= gi - 9:
                        v1_group(v1n)
                        v1n += 1
                # tail: finish d2, sqrt rest (ungates remaining v1)
                for cc in range(d2n, 16):
                    d2_group(cc)
                nc.vector.tensor_scalar(vn[:, 67:70, :], vn[:, 67:70, :],
                                        mt[:, 1:2], None, AL.mult)
                nc.scalar.activation(vn[:, 52:70, :], vn[:, 52:70, :],
                                     AF.Sqrt)
            xps_cm.__exit__(None, None, None)

            # ------------- vn W pass + integration -------------
            with tc.tile_pool(name="vps", bufs=3,
                              space=bass.MemorySpace.PSUM) as vps, \
                 tc.tile_pool(name="mps", bufs=2,
                              space=bass.MemorySpace.PSUM) as mps, \
                 tc.tile_pool(name="ivp", bufs=1,
                              space=bass.MemorySpace.PSUM) as ivp:
                ivq = ivp.tile([128, 512], F32, tag="rps", name="ivq")
                nc.vector.memset(ivq[0:65, :], 0.0)
                for cc in range(16):
                    if v1n < 16:
                        v1_group(v1n)
                        v1n += 1
                    g4 = 4 * cc
                    vq = vps.tile([128, 4, 128], F32, tag="vq", name="vq")
                    smooth_group("v2", cc, s1, ct["KI"], vq, "W")
                    mq = mps.tile([128, 4, 128], F32, tag="mq", name="mq")
                    nc.tensor.matmul(mq[:], ct["MCTC"],
                                     bcv[:, g4:g4 + 4, :],
                                     start=True, stop=True)
                    mq_sb = tzp.tile([128, 4, 128], BF16, tag="mqsb",
                                     name="mq_sb")
                    nc.scalar.activation(mq_sb[:], mq[:], AF.Identity,
                                         bias=mt[:, 2:3])
                    pchunk = tzp.tile([128, 4, 128], BF16, tag="pchunk",
                                      name="pchunk")
                    nc.vector.tensor_mul(pchunk[:], mq_sb[:], vq[:])
                    T, r = cc // 3, cc % 3
                    nc.tensor.matmul(ivq[32 * r:32 * r + 1, :], ct["ONESC"],
                                     pchunk[:], start=True, stop=True)
                    if r == 2 or cc == 15:
                        nc.vector.tensor_scalar(iv_sb[0:65, T, :],
                                                ivq[0:65, :],
                                                1.0, 0.0, AL.min, AL.max)
                for T in range(6):
                    rows = min(3, 16 - 3 * T)
                    nc.sync.dma_start(out_t[3 * T:3 * T + rows, :],
                                      iv_sb[0:32 * (rows - 1) + 1:32, T, :])
            smp_cm.__exit__(None, None, None)

    nc.compile()
    return nc


def _pack_consts():
    cm = CONSTS
    csm = np.concatenate([cm["KBD"], cm["KMD"], cm["KI"]], axis=1).astype(NPBF)
    ctz = np.concatenate([cm["MCTC"], cm["E127"], cm["ONESC"]],
                         axis=1).astype(NPBF)
    return cm["C8"], csm, ctz


def host_prepare(d_np, v_np):
    cores = []
    c8, csm, ctz = _pack_consts()
    vext = np.zeros((3, 128, 135, 129), np.float32)
    for c in range(8):
        b, hh = c // 2, c % 2
        h0 = 64 * hh
        dpad = np.zeros((128, 70, 128), np.float32)
        lo, hi = h0 - 3, h0 + 67
        src_lo, src_hi = max(lo, 0), min(hi, 128)
        dpad[:, (src_lo - lo):(src_hi - lo), :] = \
            d_np[b, 0, :, src_lo:src_hi, :]
        vext[:] = 0.0
        vext[:, :, 3:131, 0:128] = v_np[b]
        vext[:, :, 131, 0:128] = 2 * v_np[b, :, :, 127, :] - v_np[b, :, :, 126, :]
        vext[:, :, :, 128] = 2 * vext[:, :, :, 127] - vext[:, :, :, 126]
        vin = np.ascontiguousarray(vext[:, :, h0:h0 + 71, :]).astype(NPF8)
        mm = np.zeros((128, 4), np.float32)
        mm[:, 0] = 0.0 if hh == 0 else 1.0
        mm[:, 1] = 0.0 if hh == 1 else 1.0
        mm[127, 2] = 1.0
        m = {
            "d_in": dpad.astype(NPBF), "v_in": vin, "m_in": mm,
            "c8_in": c8, "csm_in": csm, "ctz_in": ctz,
        }
        cores.append(m)
    return cores


_NC = None


def kernel(d, v):
    global _NC
    d = np.asarray(d, np.float32)
    v = np.asarray(v, np.float32)
    if _NC is None:
        _NC = build_program()
    in_maps = host_prepare(d, v)
    res = run_bass_kernel_spmd(_NC, in_maps, list(range(8)))
    out = np.zeros((4, 1, 128, 128), np.float32)
    for c in range(8):
        b, hh = c // 2, c % 2
        out[b, 0, 64 * hh:64 * hh + 64, :] = \
            res.results[c]["out"].reshape(64, 128)
    return out
